# revision 9
# baseline (speedup 1.0000x reference)
"""Cayley rotation kernel for Trainium2 (8 NeuronCores, SPMD over classes).

Math per class c (D=256):
  S = 0.5*(U - U^T), U = strict-upper scatter of W[c]
  R = (I-S)(I+S)^{-1};  out[:,c] = R @ x[:,c]
Series form used on device (T = S^2, ||T|| <= ~0.123):
  out = x + 2*sum_{k>=1} (-1)^k S^k x
  q  = (x - Sx) + T(x - Sx);  u = 2(q + T^2 q);  out = (u - x) + 2 T^4 q
  (= Neumann series in T through degree 5; truncation ~3e-6 relative)
Passes (per class, fp32r matmuls; all 256-vectors live as [2 k-tiles x 128]):
  P1: stat S,  mov [S | x]      -> psum [-T | -Sx]
  P2: stat T,  mov [T | sx | x] -> psum [T2 | tsx | tx]
  DVE: q' = 2q = 2(x + tx - sx - tsx)
  P3: stat T2, mov [tsx tx sx x q'] (N=5) -> [t3sx t3x t2sx t2x T2q']
  DVE: u = q' + T2q';  ccol = u - x
  P4: (stat T2q', mov T2) + (stat ccol, mov I256) accumulated -> ps_row [1,256]
  (row = ccol^T + (T^4 q')^T = out^T for this class)

The skew matrices are shipped pre-scattered from the host (a pure
relayout/negation of W, built with one vectorized scatter), so the device
S-load is a single line-rate DMA per quarter.
"""

import numpy as np

D = 256
C = 512
NGF = 32640
NCORES = 8
CPC = C // NCORES          # 64 classes per core
QC = 16                    # classes per scatter/compute quarter
NQ = CPC // QC
BLK = 288                  # floats per (class, kt) block in the S tile (256 data + pad)
PAIR = 2 * BLK

_TRIU = np.triu_indices(D, k=1)

_CACHED = {}


def _host_prepare(x: np.ndarray, W: np.ndarray):
    """Per-core inputs: dense skew matrices (relayout+negate of W), x columns,
    identity constant."""
    S_all = np.zeros((C, D, D), np.float32)
    Wh = (0.5 * W).astype(np.float32)
    S_all[:, _TRIU[0], _TRIU[1]] = Wh
    S_all[:, _TRIU[1], _TRIU[0]] = -Wh

    ident = np.zeros((128, 512), np.float32)
    for kt in range(2):
        ident[np.arange(128), kt * 256 + kt * 128 + np.arange(128)] = 1.0

    ins = []
    for core in range(NCORES):
        c0 = core * CPC
        ws = np.ascontiguousarray(S_all[c0:c0 + CPC].reshape(CPC, D * D))
        xs = x[:, c0:c0 + CPC].reshape(2, 128, CPC)        # [kt, p, c]
        xs2 = np.ascontiguousarray(np.transpose(xs, (1, 2, 0)).reshape(128, 2 * CPC))
        ins.append({"ws": ws, "xs2": xs2.astype(np.float32), "ident": ident})
    return ins


def _build():
    if "nc" in _CACHED:
        return _CACHED["nc"]
    import concourse.bacc as bacc
    import concourse.mybir as mybir
    from concourse import tile

    f32 = mybir.dt.float32
    f32r = mybir.dt.float32r

    nc = bacc.Bacc("TRN2", target_bir_lowering=False, debug=False, num_devices=NCORES)
    ws = nc.dram_tensor("ws", [CPC, D * D], f32r, kind="ExternalInput").ap()
    xs2 = nc.dram_tensor("xs2", [128, 2 * CPC], f32r, kind="ExternalInput").ap()
    ident = nc.dram_tensor("ident", [128, 512], f32r, kind="ExternalInput").ap()
    out = nc.dram_tensor("out", [CPC, D], f32, kind="ExternalOutput").ap()

    with tile.TileContext(nc) as tc:
        with tc.tile_pool(name="sq", bufs=2) as sq_pool, \
             tc.tile_pool(name="tb", bufs=2) as t_pool, \
             tc.tile_pool(name="t2b", bufs=2) as t2_pool, \
             tc.tile_pool(name="small", bufs=1) as small, \
             tc.tile_pool(name="ps1p", bufs=1, space="PSUM") as ps1p, \
             tc.tile_pool(name="ps2p", bufs=1, space="PSUM") as ps2p, \
             tc.tile_pool(name="ps3p", bufs=2, space="PSUM") as ps3p, \
             tc.tile_pool(name="psrp", bufs=2, space="PSUM") as psrp:

            xbuf = small.tile([128, 2 * CPC], f32r)
            nc.sync.dma_start(out=xbuf[:], in_=xs2[:])
            ibuf = small.tile([128, 512], f32r)
            nc.sync.dma_start(out=ibuf[:], in_=ident[:])

            for q in range(NQ):
                cq0 = q * QC
                st = sq_pool.tile([128, QC * PAIR], f32r, tag="st")
                stage = small.tile([1, QC * D], f32, tag="stage")

                # ---- S load: one DMA; dest [p][c][kt][256] ----
                dst = st[:, :].rearrange("p (c k f) -> p c k f", k=2, f=BLK)[:, :, :, 0:256]
                src = ws[cq0:cq0 + QC, :].rearrange("c (k p f) -> p c k f", k=2, f=256)
                nc.sync.dma_start(out=dst, in_=src)

                # ---- x column into pad col 256 of each (class, kt) block ----
                stv = st[:, :].rearrange("p (m f) -> p m f", f=BLK)
                xsrc = xbuf[:, 2 * cq0: 2 * cq0 + 2 * QC].rearrange("p (c k) -> p c k", k=1)
                nc.vector.tensor_copy(stv[:, :, 256:257], xsrc)
                nc.vector.tensor_copy(stv[:, :, 257:258], xsrc)

                # ---- per-class compute ----
                for ci in range(QC):
                    base = ci * PAIR
                    sblk = [st[:, base: base + 258], st[:, base + BLK: base + BLK + 258]]

                    # P1: psum[mt*512 ..] = S^T @ [S | x] = [-T | -sx]
                    ps1 = ps1p.tile([128, 1024], f32, tag="ps1")
                    for mt in range(2):
                        for kt in range(2):
                            lhsT = st[:, base + kt * BLK + mt * 128: base + kt * BLK + mt * 128 + 128]
                            nc.tensor.matmul(ps1[:, mt * 512: mt * 512 + 258],
                                             lhsT, sblk[kt],
                                             start=(kt == 0), stop=(kt == 1))
                    # evac1 (ACT, scale=-1): tb cols 0..256 = [T | sx]
                    tb = t_pool.tile([128, PAIR], f32r, tag="tb")
                    src1 = ps1.rearrange("p (m f) -> p m f", f=512)[:, :, 0:257]
                    dstT = tb.rearrange("p (m f) -> p m f", f=BLK)[:, :, 0:257]
                    nc.scalar.mul(dstT, src1, -1.0)
                    # x col into tb col 257
                    xdst1 = tb.rearrange("p (m f) -> p m f", f=BLK)[:, :, 257:258]
                    xsrc1 = xbuf[:, 2 * (cq0 + ci): 2 * (cq0 + ci) + 2].rearrange("p (c k) -> p c k", k=1)
                    nc.vector.tensor_copy(xdst1, xsrc1)

                    # P2: psum = T @ [T | sx | x] = [T2 | tsx | tx]
                    ps2 = ps2p.tile([128, 1024], f32, tag="ps2")
                    tblk = [tb[:, 0:258], tb[:, BLK: BLK + 258]]
                    for mt in range(2):
                        for kt in range(2):
                            lhsT = tb[:, kt * BLK + mt * 128: kt * BLK + mt * 128 + 128]
                            nc.tensor.matmul(ps2[:, mt * 512: mt * 512 + 258],
                                             lhsT, tblk[kt],
                                             start=(kt == 0), stop=(kt == 1))
                    # evac2 (DVE): t2b cols 0..257 = [T2 | tsx | tx]
                    t2b = t2_pool.tile([128, PAIR], f32r, tag="t2b")
                    src2 = ps2.rearrange("p (m f) -> p m f", f=512)[:, :, 0:258]
                    dst2 = t2b.rearrange("p (m f) -> p m f", f=BLK)[:, :, 0:258]
                    nc.vector.tensor_copy(dst2, src2)
                    # cols 258,259 = [sx, x] from tb cols 256,257
                    t2v = t2b.rearrange("p (m f) -> p m f", f=BLK)
                    tbv = tb.rearrange("p (m f) -> p m f", f=BLK)
                    nc.vector.tensor_copy(t2v[:, :, 258:260], tbv[:, :, 256:258])

                    # combine: a = [tx, x] - [tsx, sx] -> cols 261,262
                    nc.vector.tensor_sub(t2v[:, :, 261:263], t2v[:, :, 257:260:2], t2v[:, :, 256:259:2])
                    # q2 = a0 + a1 -> 263 ; q' = 2*q2 -> 260
                    nc.vector.tensor_add(t2v[:, :, 263:264], t2v[:, :, 261:262], t2v[:, :, 262:263])
                    nc.vector.tensor_add(t2v[:, :, 260:261], t2v[:, :, 263:264], t2v[:, :, 263:264])

                    # P3: stat T2, mov cols 256:261 = [tsx tx sx x q'] (N=5)
                    ps3 = ps3p.tile([128, 32], f32, tag="ps3")
                    for mt in range(2):
                        for kt in range(2):
                            lhsT = t2b[:, kt * BLK + mt * 128: kt * BLK + mt * 128 + 128]
                            nc.tensor.matmul(ps3[:, mt * 16: mt * 16 + 6],
                                             lhsT, t2v[:, kt, 256:262],
                                             start=(kt == 0), stop=(kt == 1))
                    # evac3 (DVE): cols 264..268 = [t3sx t3x t2sx t2x T2q']
                    src3 = ps3.rearrange("p (m f) -> p m f", f=16)[:, :, 0:5]
                    nc.vector.tensor_copy(t2v[:, :, 264:269], src3)

                    # u = q' + T2q' (269) ; ccol = u - x (270)
                    nc.vector.tensor_add(t2v[:, :, 269:270], t2v[:, :, 260:261], t2v[:, :, 268:269])
                    nc.vector.tensor_sub(t2v[:, :, 270:271], t2v[:, :, 269:270], t2v[:, :, 259:260])

                    # P4: ps_row[1,256] = (T2q')^T @ T2  +  ccol^T @ I
                    psr = psrp.tile([1, 512], f32, tag="psr")
                    first = True
                    for kt in range(2):
                        nc.tensor.matmul(psr[0:1, 0:256],
                                         t2v[:, kt, 268:269],
                                         t2b[:, kt * BLK: kt * BLK + 256],
                                         start=first, stop=False)
                        first = False
                        nc.tensor.matmul(psr[0:1, 0:256],
                                         t2v[:, kt, 270:271],
                                         ibuf[:, kt * 256: kt * 256 + 256],
                                         start=False, stop=(kt == 1))
                    nc.vector.tensor_copy(stage[0:1, ci * D:(ci + 1) * D], psr[0:1, 0:256])

                # per-quarter output DMA
                nc.sync.dma_start(out=out[cq0:cq0 + QC, :].unsqueeze(0),
                                  in_=stage[0:1, :].rearrange("p (c f) -> p c f", f=D))

    nc.compile()
    _CACHED["nc"] = nc
    return nc


def kernel(x: np.ndarray, W: np.ndarray) -> np.ndarray:
    from concourse.bass_utils import run_bass_kernel_spmd
    nc = _build()
    ins = _host_prepare(np.asarray(x, np.float32), np.asarray(W, np.float32))
    res = run_bass_kernel_spmd(nc, ins, list(range(NCORES)), trace=False)
    parts = [res.results[core]["out"] for core in range(NCORES)]
    full = np.concatenate(parts, axis=0)  # (C, D)
    return np.ascontiguousarray(full.T).astype(np.float32)  # (D, C)
